# revision 9
# baseline (speedup 1.0000x reference)
"""Multi-head attention (S=4096, D=1024, H=16, dk=dv=64) on 8 trn2 NeuronCores.

Sharding: tensor-parallel over heads - 2 heads per core. Each core receives
the (host-transposed, bf16-cast) activations plus its two heads' projection
weights and its 128-column slice of Wo, computes its heads' attention and a
partial output product y_c = hc @ Wo[:, c-slice].T; the host all-reduces the
8 partials (row-shard W_o, all-reduce in the gather step).

Per-core kernel, ACT(exp)-paced: the softmax exp over 2*S*S scores is the
irreducible bottleneck (~1us per [128,1024] PSUM-sourced ACTIVATE, 256 of
them). Everything else is scheduled under that pace:

  - 8 logical sq strips of 512. Strips 0+1 run t-interleaved as one double
    phase so all k/v projections (which every strip needs) amortize under
    ~68us of exp time; q projections drip one strip ahead.
  - Per window (strip s, sk tile t): QK for the *next* window (row-tiled,
    both heads concurrent at base partitions 0/64), one exp over [128,1024]
    (scale=1/8, no max subtraction - scores are O(1)), then PV col-tiled
    (head A -> psum rows 0-63, head B -> 64-127, concurrent) and two M=1
    denominator matmuls (ones-vector lhsT) into a shared den bank at rows
    {0,64} ({32,96} for strip 1 in the double phase).
  - vh tiles [sk, dv] come from SBUF->SBUF xbar DMA transposes (off the PE).
  - Strip epilogue (dripped into the next strip's windows): fp32
    reciprocal_approx_fast on the den rows, one K=65/97 fp32 selector
    matmul broadcasting 1/den across partitions, one in-place normalize
    mult, then 8 output-projection matmuls; the last strip's epilogue
    borrows the dead qk PSUM slots and splits copies across ACT+DVE.

Matmul operands are bf16; PSUM fp32; exp input exact fp32; denominators fp32
through the reciprocal. PSUM banks: qk 2x[128,1024]=4, pv 2x[128,512]=2,
den 1, post 1 = 8.
"""

import contextlib
import sys

if "/opt/trn_rl_repo" not in sys.path:
    sys.path.insert(0, "/opt/trn_rl_repo")

import numpy as np
import ml_dtypes

import concourse.bass as bass
import concourse.mybir as mybir
import concourse.tile as tile
from concourse.masks import make_identity

F32 = mybir.dt.float32
BF16 = mybir.dt.bfloat16
EXP = mybir.ActivationFunctionType.Exp
MULT = mybir.AluOpType.mult

S, D, DK, P, W = 4096, 1024, 64, 128, 512
NS = S // W      # 8 logical sq strips
NT = S // P      # 32 sk tiles
NDC = D // P     # 8 contraction chunks for projections
TPW = W // P     # sq tiles per strip (4)
SCALE = 0.125    # 1/sqrt(DK)
NCORES = 8

# denominator rows in the shared den PSUM bank (col-group positions)
DEN_ROWS = {s: (0, 64) for s in range(NS)}
DEN_ROWS[1] = (32, 96)  # strip 1 shares the bank with strip 0 in the double phase


def _split_excess_waits(nc, max_waits=1, max_waits_evsem=2):
    """The walrus build in this container rejects instructions carrying more
    than ~2 sync-wait commands; Tile's exit drain aggregates one wait per live
    semaphore onto single instructions. Split the excess onto preceding NoOps
    on the same engine (engine streams are in-order, so semantics hold)."""
    for fn in nc.m.functions:
        for blk in fn.blocks:
            new_insts = []
            for inst in blk.instructions:
                si = getattr(inst, "sync_info", None)
                lim = (
                    max_waits_evsem
                    if isinstance(inst, mybir.InstEventSemaphore)
                    else max_waits
                )
                if si is not None and si.on_wait and len(si.on_wait) > lim:
                    waits = list(si.on_wait)
                    for w in waits[:-lim]:
                        new_insts.append(
                            mybir.InstNoOp(
                                name=nc.get_next_instruction_name(),
                                engine=inst.engine,
                                bass_nofuse=True,
                                sync_info=mybir.SyncInfo(on_wait=[w], on_update=[]),
                            )
                        )
                    si.on_wait = waits[-lim:]
                new_insts.append(inst)
            blk.instructions = new_insts


def _build_mha(nc: bass.Bass):
    qT = nc.dram_tensor("qT", [D, S], BF16, kind="ExternalInput")
    kT = nc.dram_tensor("kT", [D, S], BF16, kind="ExternalInput")
    vT = nc.dram_tensor("vT", [D, S], BF16, kind="ExternalInput")
    wq = nc.dram_tensor("wq", [D, P], BF16, kind="ExternalInput")
    wk = nc.dram_tensor("wk", [D, P], BF16, kind="ExternalInput")
    wv = nc.dram_tensor("wv", [D, P], BF16, kind="ExternalInput")
    wo = nc.dram_tensor("wo", [P, D], BF16, kind="ExternalInput")
    seld = nc.dram_tensor("seld", [P, 2, P], F32, kind="ExternalInput")
    y = nc.dram_tensor("y", [S, D], BF16, kind="ExternalOutput")

    qT3 = qT.rearrange("(o p) s -> p o s", p=P)
    kT3 = kT.rearrange("(o p) s -> p o s", p=P)
    vT3 = vT.rearrange("(o p) s -> p o s", p=P)
    wq3 = wq.rearrange("(o p) m -> p o m", p=P)
    wk3 = wk.rearrange("(o p) m -> p o m", p=P)
    wv3 = wv.rearrange("(o p) m -> p o m", p=P)

    # window order: strips 0+1 interleaved per t, then strips 2..7 sequential
    windows = []
    for t in range(NT):
        windows.append((0, t))
        windows.append((1, t))
    for s in range(2, NS):
        for t in range(NT):
            windows.append((s, t))
    NWIN = len(windows)

    with tile.TileContext(nc) as tc, contextlib.ExitStack() as ctx:
        static = ctx.enter_context(tc.tile_pool(name="static", bufs=1))
        xpool = ctx.enter_context(tc.tile_pool(name="x", bufs=4))
        vtmp = ctx.enter_context(tc.tile_pool(name="vtmp", bufs=2))
        ptp = ctx.enter_context(tc.tile_pool(name="pt", bufs=4))
        hcp = ctx.enter_context(tc.tile_pool(name="hc", bufs=3))
        recp = ctx.enter_context(tc.tile_pool(name="rec", bufs=2))
        ysp = ctx.enter_context(tc.tile_pool(name="ys", bufs=4))
        qkp = ctx.enter_context(tc.tile_pool(name="qkps", bufs=2, space="PSUM"))
        pvp = ctx.enter_context(tc.tile_pool(name="pvps", bufs=2, space="PSUM"))
        denp = ctx.enter_context(tc.tile_pool(name="denps", bufs=1, space="PSUM"))
        postp = ctx.enter_context(tc.tile_pool(name="postps", bufs=1, space="PSUM"))

        wq_sb = static.tile([P, NDC, P], BF16, tag="wq")
        wk_sb = static.tile([P, NDC, P], BF16, tag="wk")
        wv_sb = static.tile([P, NDC, P], BF16, tag="wv")
        wo_sb = static.tile([P, D], BF16, tag="wo")
        ident = static.tile([P, P], BF16, tag="ident")
        sel = static.tile([P, 2, P], F32, tag="sel")
        ones2 = static.tile([P, 2], BF16, tag="ones2")
        khT = static.tile([P, S], BF16, tag="khT")
        qhT = static.tile([P, S], BF16, tag="qhT")
        vh = static.tile([P, NT, P], BF16, tag="vh")

        nc.sync.dma_start(wq_sb[:], wq3)
        nc.sync.dma_start(wk_sb[:], wk3)
        nc.sync.dma_start(wv_sb[:], wv3)
        nc.sync.dma_start(sel[:], seld[:])
        nc.sync.dma_start(wo_sb[:], wo[:])
        make_identity(nc, ident[:])
        nc.gpsimd.memset(ones2[:], 1.0)

        # the den bank's non-denominator rows are read by the reciprocal and
        # the selector matmul (zero-weighted, but must stay finite): set the
        # whole bank to 1.0 once; den matmuls only ever overwrite their rows
        den_init = denp.tile([P, W], F32, tag="den")
        nc.vector.memset(den_init[:], 1.0)

        # HAM warm-up: ~4us of dummy PE work (no DMA dependency) so the
        # projections hit the array already at 2.4 GHz.
        warm = postp.tile([P, W], F32, tag="post")
        for _ in range(40):
            nc.tensor.matmul(warm[:, 0:P], ident[:], ident[:], start=True, stop=True)

        # ---------- projection / DMA thunk builders ----------
        def dma_xx_thunk(src3, c):
            box = []

            def _th():
                xx = xpool.tile([P, NDC, W], BF16, tag="xs")
                nc.sync.dma_start(xx[:], src3[:, :, c * W : (c + 1) * W])
                box.append(xx)

            return _th, box

        def proj_half_thunks(w_sb, xx_box, dst):
            """One projection chunk as 2 PE thunks of 4 matmuls (dst bf16)."""
            pp_box = []

            def h1():
                pp = postp.tile([P, W], F32, tag="post")
                pp_box.append(pp)
                for c in range(4):
                    nc.tensor.matmul(
                        pp[:], w_sb[:, c, :], xx_box[0][:, c, :],
                        start=(c == 0), stop=False,
                    )

            def h2():
                pp = pp_box[0]
                for c in range(4, NDC):
                    nc.tensor.matmul(
                        pp[:], w_sb[:, c, :], xx_box[0][:, c, :],
                        start=False, stop=(c == NDC - 1),
                    )
                nc.vector.tensor_copy(dst, pp[:])

            return [h1, h2]

        def v_chunk_thunks(c):
            """v projection chunk c -> vts -> xbar-transposed vh tiles."""
            dma_th, xx_box = dma_xx_thunk(vT3, c)
            vts_box = []
            pp_box = []

            def h1():
                pp = postp.tile([P, W], F32, tag="post")
                pp_box.append(pp)
                for cc in range(4):
                    nc.tensor.matmul(
                        pp[:], wv_sb[:, cc, :], xx_box[0][:, cc, :],
                        start=(cc == 0), stop=False,
                    )

            def h2():
                pp = pp_box[0]
                for cc in range(4, NDC):
                    nc.tensor.matmul(
                        pp[:], wv_sb[:, cc, :], xx_box[0][:, cc, :],
                        start=False, stop=(cc == NDC - 1),
                    )
                vts = vtmp.tile([P, W], BF16, tag="vts")
                vts_box.append(vts)
                nc.vector.tensor_copy(vts[:], pp[:])

            def trs():
                vts = vts_box[0]
                for i in range(TPW):
                    nc.sync.dma_start(
                        vh[:, c * TPW + i, :],
                        vts[:, i * P : (i + 1) * P],
                        transpose=True,
                    )

            return dma_th, [h1, h2, trs]

        def q_chunk_thunks(c):
            dma_th, xx_box = dma_xx_thunk(qT3, c)
            return dma_th, proj_half_thunks(wq_sb, xx_box, qhT[:, c * W : (c + 1) * W])

        def k_chunk_thunks(c):
            dma_th, xx_box = dma_xx_thunk(kT3, c)
            return dma_th, proj_half_thunks(wk_sb, xx_box, khT[:, c * W : (c + 1) * W])

        # ---------- attention emission ----------
        winqk = {}

        def emit_qk(i):
            s, t = windows[i]
            qk = qkp.tile([P, 2 * W], F32, tag="qk")
            cw = slice(s * W, (s + 1) * W)
            ts = slice(t * P, (t + 1) * P)
            nc.tensor.matmul(
                qk[:, 0:W], khT[0:DK, ts], qhT[0:DK, cw], start=True, stop=True
            )
            nc.tensor.matmul(
                qk[:, W : 2 * W], khT[DK:P, ts], qhT[DK:P, cw], start=True, stop=True
            )
            winqk[i] = qk

        pv_acc = {}
        den_t = {}

        def emit_pv_den(s, t, pt):
            if t == 0:
                if s == 1:
                    den_t[1] = den_t[0]  # double phase shares the bank
                else:
                    den_t[s] = denp.tile([P, W], F32, tag="den", name=f"den{s}")
                pv_acc[s] = pvp.tile([P, W], F32, tag="pv", name=f"pv{s}")
            pv = pv_acc[s]
            first, last = (t == 0), (t == NT - 1)
            nc.tensor.matmul(
                pv[0:DK, :], vh[:, t, 0:DK], pt[:, 0:W], start=first, stop=last
            )
            nc.tensor.matmul(
                pv[DK:P, :], vh[:, t, DK:P], pt[:, W : 2 * W], start=first, stop=last
            )
            dt_ = den_t[s]
            ra, rb = DEN_ROWS[s]
            nc.tensor.matmul(
                dt_[ra : ra + 1, :], ones2[:, 0:1], pt[:, 0:W],
                start=first, stop=last, tile_position=(0, ra),
            )
            nc.tensor.matmul(
                dt_[rb : rb + 1, :], ones2[:, 1:2], pt[:, W : 2 * W],
                start=first, stop=last, tile_position=(0, rb),
            )

        hc_t = {}
        rec_t = {}

        def strip_end(s):
            """Boundary work emitted immediately: evacuate pv, recip the dens.
            Strip 0's recip is deferred to strip 1's boundary — the shared den
            bank's row 32 is still accumulating strip 1's denominator."""
            hc = hcp.tile([P, W], BF16, tag="hc", name=f"hc{s}")
            if s == NS - 1:
                nc.scalar.copy(hc[:], pv_acc[s][:])  # ACT is idle in the tail
            else:
                nc.vector.tensor_copy(hc[:], pv_acc[s][:])
            hc_t[s] = hc
            if s == 0:
                return
            if s == 1:
                rec0 = recp.tile([P, W], F32, tag="rec", name="rec0")
                nc.vector.reciprocal(rec0[0:65, :], den_t[0][0:65, :])
                rec_t[0] = rec0
            rec = recp.tile([P, W], F32, tag="rec", name=f"rec{s}")
            rk = 97 if s == 1 else 65
            nc.vector.reciprocal(rec[0:rk, :], den_t[s][0:rk, :])
            rec_t[s] = rec

        def make_epilogue(s, hc, rec, last=False):
            sidx = 1 if s == 1 else 0
            K = 97 if s == 1 else 65
            thunks = []

            def bcast_mult():
                bc = postp.tile([P, W], F32, tag="post")
                nc.tensor.matmul(
                    bc[:], sel[0:K, sidx, :], rec[0:K, :], start=True, stop=True
                )
                nc.vector.tensor_tensor(hc[:], hc[:], bc[:], op=MULT)

            thunks.append(bcast_mult)

            for i in range(TPW):
                for oh in range(2):
                    def po(i=i, oh=oh):
                        if last:
                            py = qkp.tile([P, W], F32, tag="qk")
                        else:
                            py = postp.tile([P, W], F32, tag="post")
                        nc.tensor.matmul(
                            py[:],
                            hc[:, i * P : (i + 1) * P],
                            wo_sb[:, oh * W : (oh + 1) * W],
                            start=True, stop=True,
                        )
                        ys = ysp.tile([P, W], BF16, tag="ys")
                        if last and oh == 1:
                            nc.scalar.copy(ys[:], py[:])
                        else:
                            nc.vector.tensor_copy(ys[:], py[:])
                        nc.sync.dma_start(
                            y[(s * TPW + i) * P : (s * TPW + i + 1) * P,
                              oh * W : (oh + 1) * W],
                            ys[:],
                        )

                    thunks.append(po)
            return thunks

        # ---------- prologue ----------
        dq0, q0_th = q_chunk_thunks(0)
        dq1, q1_th = q_chunk_thunks(1)
        dk0, k0_th = k_chunk_thunks(0)
        dv0, v0_th = v_chunk_thunks(0)
        dq0(); dk0(); dv0(); dq1()
        for th in q0_th:
            th()
        for th in k0_th:
            th()
        for th in v0_th:
            th()
        for th in q1_th:
            th()
        emit_qk(0)

        # drip lists
        phase0 = []
        for c in range(1, NDC):
            dk_th, kth = k_chunk_thunks(c)
            dv_th, vth = v_chunk_thunks(c)
            phase0 += [dk_th, dv_th] + kth + vth
        dq2, q2_th = q_chunk_thunks(2)
        phase0 += [dq2] + q2_th

        pending = list(phase0)
        strip_q = {}  # s -> thunks for q chunk of strip s+1, dripped in strip s
        for s in range(2, NS - 1):
            dq, qth = q_chunk_thunks(s + 1)
            strip_q[s] = [dq] + qth

        epi = {}  # s -> epilogue thunks

        # ---------- main loop ----------
        pend_wait = 0
        for i, (s, t) in enumerate(windows):
            qk = winqk.pop(i)
            pt = ptp.tile([P, 2 * W], BF16, tag="pt")
            nc.scalar.activation(pt[:], qk[:], EXP, scale=SCALE)
            if i + 1 < NWIN:
                emit_qk(i + 1)
            # drip (before PV so the PE fills its exp-wait slack with it)
            if pending and pend_wait == 0:
                pending.pop(0)()
            elif pend_wait > 0:
                pend_wait -= 1
            emit_pv_den(s, t, pt)
            if t == NT - 1:
                strip_end(s)
                if s == 1:
                    assert not pending, f"phase0 drip leftover: {len(pending)}"
                    epi[0] = make_epilogue(0, hc_t[0], rec_t[0])
                    epi[1] = make_epilogue(1, hc_t[1], rec_t[1])
                    pending = epi[0] + epi[1] + strip_q.get(2, [])
                    pend_wait = 4
                elif s >= 2:
                    assert not pending, f"strip {s} drip leftover: {len(pending)}"
                    epi[s] = make_epilogue(
                        s, hc_t[s], rec_t[s], last=(s == NS - 1)
                    )
                    if s < NS - 1:
                        pending = epi[s] + strip_q.get(s + 1, [])
                        pend_wait = 4

        # ---------- tail: last strip's epilogue ----------
        for th in epi[NS - 1]:
            th()
    return nc


def _make_core_inputs(q, k, v, Wq, Wk, Wv, Wo, core, cache):
    bf = ml_dtypes.bfloat16
    if "qT" not in cache:
        cache["qT"] = np.ascontiguousarray(q.T).astype(bf)
        cache["kT"] = np.ascontiguousarray(k.T).astype(bf)
        cache["vT"] = np.ascontiguousarray(v.T).astype(bf)
        seld = np.zeros((P, 2, P), np.float32)
        seld[0, 0, 0:DK] = 1.0
        seld[DK, 0, DK:P] = 1.0
        seld[32, 1, 0:DK] = 1.0
        seld[96, 1, DK:P] = 1.0
        cache["seld"] = seld
    h0, h1 = 2 * core, 2 * core + 1
    return {
        "qT": cache["qT"],
        "kT": cache["kT"],
        "vT": cache["vT"],
        "wq": np.concatenate([Wq[h0], Wq[h1]], axis=1).astype(bf),
        "wk": np.concatenate([Wk[h0], Wk[h1]], axis=1).astype(bf),
        "wv": np.concatenate([Wv[h0], Wv[h1]], axis=1).astype(bf),
        "wo": np.ascontiguousarray(Wo[:, P * core : P * (core + 1)].T).astype(bf),
        "seld": cache["seld"],
    }


_NC = None
last_results = None  # BassKernelResults of the most recent run (for profiling)


def _get_nc():
    global _NC
    if _NC is None:
        nc = bass.Bass("TRN2", target_bir_lowering=False, debug=False)
        _build_mha(nc)
        _split_excess_waits(nc)
        _NC = nc
    return _NC


def kernel(q, k, v, Wq, Wk, Wv, Wo, **run_kwargs):
    """Full-input MHA forward. Shards over 8 NeuronCores (2 heads each),
    runs the Bass kernel, and all-reduces the per-core partial outputs."""
    from concourse.bass_utils import run_bass_kernel_spmd

    global last_results
    q = np.asarray(q, np.float32)
    k = np.asarray(k, np.float32)
    v = np.asarray(v, np.float32)
    Wq = np.asarray(Wq, np.float32)
    Wk = np.asarray(Wk, np.float32)
    Wv = np.asarray(Wv, np.float32)
    Wo = np.asarray(Wo, np.float32)

    nc = _get_nc()
    cache = {}
    in_maps = [
        _make_core_inputs(q, k, v, Wq, Wk, Wv, Wo, c, cache) for c in range(NCORES)
    ]
    res = run_bass_kernel_spmd(
        nc, in_maps, core_ids=list(range(NCORES)), **run_kwargs
    )
    last_results = res
    y = res.results[0]["y"].astype(np.float32)
    for c in range(1, NCORES):
        y += res.results[c]["y"].astype(np.float32)
    return y


# revision 14
# speedup vs baseline: 1.1921x; 1.1921x over previous
"""Multi-head attention (S=4096, D=1024, H=16, dk=dv=64) on 8 trn2 NeuronCores.

Sharding: tensor-parallel over heads — 2 heads per core. Each core receives
the (host-transposed, bf16-cast) activations plus its two heads' projection
weights and its 128-column slice of Wo, computes its heads' attention and a
partial output product y_c = hc @ Wo[:, c-slice].T, and the host all-reduces
the 8 partials (the "row-shard W_o and all-reduce" variant, with the
all-reduce in the gather step).

Per-core Bass/Tile kernel (engines execute their streams in order, so the
program is software-pipelined by emission order):
  - Projections qhT/khT [128,S] (head A on partitions 0-63, head B on 64-127)
    and vh tiles [sk, dv]+ones-column (PE-transposed), streamed per 512-wide
    strip and interleaved into strip 0's attention loop (DMA-paced).
  - Attention, ACT-paced steady state: per sk tile, scores^T for both heads
    concurrently (PE row groups at base partitions 0/64), one Exp over the
    [128,1024] PSUM pair (scale=1/8; no max subtraction — scores are O(1)),
    two PV accumulations out^T[dv+1, sq]; the ones column yields the softmax
    denominator in row 64.
  - Per-strip epilogue, drip-fed into the NEXT strip's loop: one reciprocal
    over both denominator rows (adjacent partitions via an SBUF->SBUF DMA
    partition move), K=2 selector-matmul broadcast across 64 partitions
    (DVE cannot cross partitions), in-place normalize, head B shifted under
    head A by DMA, then single K=128 output-projection matmuls.

Matmul operands are bf16 (full PE rate; fp32 is 1/4 rate and fp32r is an
e11m8 format with the same 8-bit mantissa). PSUM accumulation is fp32; exp
input is exact fp32; denominators stay fp32 through the reciprocal.
PSUM (8 banks): qk pairs 2x[128,1024]=4, pv 2x[128,512]=2, post 2x[128,512]=2.
"""

import contextlib
import sys

if "/opt/trn_rl_repo" not in sys.path:
    sys.path.insert(0, "/opt/trn_rl_repo")

import numpy as np
import ml_dtypes

import concourse.bass as bass
import concourse.mybir as mybir
import concourse.tile as tile
from concourse.masks import make_identity

F32 = mybir.dt.float32
BF16 = mybir.dt.bfloat16
EXP = mybir.ActivationFunctionType.Exp
MULT = mybir.AluOpType.mult

S, D, DK, P, W = 4096, 1024, 64, 128, 512
NS = S // W      # 8 sq strips
NT = S // P      # 32 sk tiles
NDC = D // P     # 8 contraction chunks for projections
TPW = W // P     # sk tiles per strip (4)
SCALE = 0.125    # 1/sqrt(DK)
NCORES = 8


def _split_excess_waits(nc, max_waits=1, max_waits_evsem=2):
    """The walrus build in this container rejects instructions carrying more
    than ~2 sync-wait commands; Tile's exit drain aggregates one wait per live
    semaphore onto single instructions. Split the excess onto preceding NoOps
    on the same engine (engine streams are in-order, so semantics hold)."""
    for fn in nc.m.functions:
        for blk in fn.blocks:
            new_insts = []
            for inst in blk.instructions:
                si = getattr(inst, "sync_info", None)
                lim = (
                    max_waits_evsem
                    if isinstance(inst, mybir.InstEventSemaphore)
                    else max_waits
                )
                if si is not None and si.on_wait and len(si.on_wait) > lim:
                    waits = list(si.on_wait)
                    for w in waits[:-lim]:
                        new_insts.append(
                            mybir.InstNoOp(
                                name=nc.get_next_instruction_name(),
                                engine=inst.engine,
                                bass_nofuse=True,
                                sync_info=mybir.SyncInfo(on_wait=[w], on_update=[]),
                            )
                        )
                    si.on_wait = waits[-lim:]
                new_insts.append(inst)
            blk.instructions = new_insts


def _build_mha(nc: bass.Bass):
    qT = nc.dram_tensor("qT", [D, S], BF16, kind="ExternalInput")
    kT = nc.dram_tensor("kT", [D, S], BF16, kind="ExternalInput")
    vT = nc.dram_tensor("vT", [D, S], BF16, kind="ExternalInput")
    wq = nc.dram_tensor("wq", [D, P], BF16, kind="ExternalInput")
    wk = nc.dram_tensor("wk", [D, P], BF16, kind="ExternalInput")
    wv = nc.dram_tensor("wv", [D, P], BF16, kind="ExternalInput")
    wo = nc.dram_tensor("wo", [P, D], BF16, kind="ExternalInput")
    seld = nc.dram_tensor("seld", [2, 2 * DK], BF16, kind="ExternalInput")
    y = nc.dram_tensor("y", [S, D], BF16, kind="ExternalOutput")

    qT3 = qT.rearrange("(o p) s -> p o s", p=P)
    kT3 = kT.rearrange("(o p) s -> p o s", p=P)
    vT3 = vT.rearrange("(o p) s -> p o s", p=P)
    wq3 = wq.rearrange("(o p) m -> p o m", p=P)
    wk3 = wk.rearrange("(o p) m -> p o m", p=P)
    wv3 = wv.rearrange("(o p) m -> p o m", p=P)

    with tile.TileContext(nc) as tc, contextlib.ExitStack() as ctx:
        static = ctx.enter_context(tc.tile_pool(name="static", bufs=1))
        xpool = ctx.enter_context(tc.tile_pool(name="x", bufs=8))
        vtmp = ctx.enter_context(tc.tile_pool(name="vtmp", bufs=2))
        ptp = ctx.enter_context(tc.tile_pool(name="pt", bufs=6))
        recp = ctx.enter_context(tc.tile_pool(name="rec", bufs=2))
        ystage = ctx.enter_context(tc.tile_pool(name="ystage", bufs=4))
        qk_ps = ctx.enter_context(tc.tile_pool(name="qkps", bufs=2, space="PSUM"))
        pv_ps = ctx.enter_context(tc.tile_pool(name="pvps", bufs=2, space="PSUM"))
        post_ps = ctx.enter_context(tc.tile_pool(name="postps", bufs=2, space="PSUM"))

        wq_sb = static.tile([P, NDC, P], BF16, tag="wq")
        wk_sb = static.tile([P, NDC, P], BF16, tag="wk")
        wv_sb = static.tile([P, NDC, P], BF16, tag="wv")
        wo_sb = static.tile([P, D], BF16, tag="wo")
        ident = static.tile([P, P], BF16, tag="ident")
        sel = static.tile([DK + 2, 2 * DK], BF16, tag="sel")
        khT = static.tile([P, S], BF16, tag="khT")
        qhT = static.tile([P, S], BF16, tag="qhT")
        vh = static.tile([P, NT, 2 * DK + 2], BF16, tag="vh")
        hc = static.tile([P, S], BF16, tag="hc")
        tmpb_pool = recp

        nc.sync.dma_start(wq_sb[:], wq3)
        nc.sync.dma_start(wk_sb[:], wk3)
        nc.sync.dma_start(wv_sb[:], wv3)
        make_identity(nc, ident[:])
        # HAM warm-up: ~4us of dummy PE work (no DMA dependency) so the
        # projections hit the array already at 2.4 GHz.
        warm = post_ps.tile([P, W], F32, tag="post")
        for _ in range(40):
            nc.tensor.matmul(warm[:, 0:P], ident[:], ident[:], start=True, stop=True)

        def one_proj(jw, w_sb, src3, dst):
            def _th():
                xx = xpool.tile([P, NDC, W], BF16, tag="xs")
                nc.sync.dma_start(xx[:], src3[:, :, jw])
                pp = post_ps.tile([P, W], F32, tag="post")
                for c in range(NDC):
                    nc.tensor.matmul(
                        pp[:], w_sb[:, c, :], xx[:, c, :],
                        start=(c == 0), stop=(c == NDC - 1),
                    )
                nc.vector.tensor_copy(dst, pp[:])
            return _th

        def proj_thunks(j):
            """Projection strip j as 4 thunks: q, k, v, v-transposes."""
            jw = slice(j * W, (j + 1) * W)
            vts = vtmp.tile([P, W], BF16, tag="vts")

            def _trs():
                for i in range(TPW):
                    t = j * TPW + i
                    ptr = post_ps.tile([P, P], BF16, tag="post")
                    nc.tensor.transpose(ptr[:], vts[:, i * P : (i + 1) * P], ident[:])
                    # ptr rows = sk; cols 0:64 head A dv, 64:128 head B dv
                    nc.vector.tensor_copy(vh[:, t, 0:DK], ptr[:, 0:DK])
                    nc.vector.tensor_copy(
                        vh[:, t, DK + 1 : 2 * DK + 1], ptr[:, DK : 2 * DK]
                    )
                    nc.gpsimd.memset(vh[:, t, DK : DK + 1], 1.0)
                    nc.gpsimd.memset(vh[:, t, 2 * DK + 1 : 2 * DK + 2], 1.0)

            return [
                one_proj(jw, wq_sb, qT3, qhT[:, jw]),
                one_proj(jw, wk_sb, kT3, khT[:, jw]),
                one_proj(jw, wv_sb, vT3, vts[:]),
                _trs,
            ]

        def emit_proj(j):
            for th in proj_thunks(j):
                th()

        def make_epilogue(s, rec, tmpb):
            """Deferred post-softmax work for strip s, drip-fed into the next
            strip's attention loop (fills PE slack under the exp pace)."""
            cw = slice(s * W, (s + 1) * W)
            thunks = []

            def norm_a():
                bc = post_ps.tile([P, W], F32, tag="post")
                nc.tensor.matmul(
                    bc[0:DK, :], sel[DK : DK + 2, 0:DK], rec[DK : DK + 2, :],
                    start=True, stop=True,
                )
                nc.vector.tensor_tensor(
                    hc[0:DK, cw], hc[0:DK, cw], bc[0:DK, :], op=MULT
                )

            def norm_b():
                bc = post_ps.tile([P, W], F32, tag="post")
                nc.tensor.matmul(
                    bc[0:DK, :], sel[DK : DK + 2, DK : 2 * DK], rec[DK : DK + 2, :],
                    start=True, stop=True,
                )
                nc.vector.tensor_tensor(
                    tmpb[0:DK, :], tmpb[0:DK, :], bc[0:DK, :], op=MULT
                )
                nc.sync.dma_start(hc[DK:P, cw], tmpb[0:DK, :])

            thunks.append(norm_a)
            thunks.append(norm_b)

            def proj_out(i, oh):
                def _th():
                    sq = s * TPW + i
                    py = post_ps.tile([P, W], F32, tag="post")
                    nc.tensor.matmul(
                        py[:],
                        hc[:, sq * P : (sq + 1) * P],
                        wo_sb[:, oh * W : (oh + 1) * W],
                        start=True, stop=True,
                    )
                    ys = ystage.tile([P, W], BF16, tag="ys")
                    # final strip: exp stream is over, ScalarE is idle — let it
                    # share the PSUM evacuations so the tail chain is shorter
                    if s == NS - 1 and oh == 1:
                        nc.scalar.copy(ys[:], py[:])
                    else:
                        nc.vector.tensor_copy(ys[:], py[:])
                    nc.sync.dma_start(
                        y[sq * P : (sq + 1) * P, oh * W : (oh + 1) * W], ys[:]
                    )
                return _th

            for i in range(TPW):
                for oh in range(2):
                    thunks.append(proj_out(i, oh))
            return thunks

        # ---- main software-pipelined loop ----
        emit_proj(0)
        nc.sync.dma_start(wo_sb[:], wo[:])
        nc.sync.dma_start(sel[DK : DK + 2, :], seld[:])
        # per-strip projection thunks in dependency order; q(1) stays in
        # strip 0's drip (its consumer starts next strip), q(2..7) drip one
        # per strip s-1 instead of piling onto strip 0's PE
        projq = []
        qdrip = {}
        for j in range(1, NS):
            q_th, k_th, v_th, trs_th = proj_thunks(j)
            projq += [k_th, v_th, trs_th]
            if j == 1:
                projq.append(q_th)
            else:
                qdrip[j - 1] = q_th

        pending = []
        for s in range(NS):
            cw = slice(s * W, (s + 1) * W)
            pva = pv_ps.tile([P, W], F32, tag="pv")
            pvb = pv_ps.tile([P, W], F32, tag="pv")
            for t in range(NT):
                if s == 0 and projq:
                    projq.pop(0)()
                qk = qk_ps.tile([P, 2 * W], F32, tag="qk")
                nc.tensor.matmul(
                    qk[0:P, 0:W],
                    khT[0:DK, t * P : (t + 1) * P], qhT[0:DK, cw],
                    start=True, stop=True,
                )
                nc.tensor.matmul(
                    qk[0:P, W : 2 * W],
                    khT[DK:P, t * P : (t + 1) * P], qhT[DK:P, cw],
                    start=True, stop=True,
                )
                pt = ptp.tile([P, 2 * W], BF16, tag="pt")
                nc.scalar.activation(pt[:], qk[:], EXP, scale=SCALE)
                nc.tensor.matmul(
                    pva[0 : DK + 1, :],
                    vh[:, t, 0 : DK + 1], pt[:, 0:W],
                    start=(t == 0), stop=(t == NT - 1),
                )
                nc.tensor.matmul(
                    pvb[0 : DK + 1, :],
                    vh[:, t, DK + 1 : 2 * DK + 2], pt[:, W : 2 * W],
                    start=(t == 0), stop=(t == NT - 1),
                )
                if pending and t >= 16 and t % 2 == 0:
                    pending.pop(0)()
                    if t == NT - 2:
                        while pending:
                            pending.pop(0)()

            # strip boundary: evacuate PSUM fast (head A + head B + the two
            # denominator rows, B's moved to the adjacent partition by DMA),
            # then one reciprocal for both heads — all off the exp path.
            # evacuate the big pv regions FIRST so the next strip's PV can
            # claim the banks before the slow reciprocal occupies the DVE
            rs = recp.tile([P, W], F32, tag="rs")
            tmpb = tmpb_pool.tile([DK, W], BF16, tag="tmpb")
            nc.vector.tensor_copy(hc[0:DK, cw], pva[0:DK, :])
            nc.vector.tensor_copy(tmpb[0:DK, :], pvb[0:DK, :])
            nc.vector.tensor_copy(rs[DK : DK + 1, :], pva[DK : DK + 1, :])
            rbt = recp.tile([P, W], F32, tag="rbt")
            nc.vector.tensor_copy(rbt[DK : DK + 1, :], pvb[DK : DK + 1, :])
            nc.sync.dma_start(rs[DK + 1 : DK + 2, :], rbt[DK : DK + 1, :])
            rec = recp.tile([P, W], BF16, tag="rec")
            with nc.allow_low_precision(
                reason="bf16 softmax denominators feed a bf16 matmul broadcast"
            ):
                nc.vector.reciprocal(rec[DK : DK + 2, :], rs[DK : DK + 2, :])

            assert not pending
            pending = make_epilogue(s, rec, tmpb)
            if (s + 1) in qdrip:
                pending.insert(0, qdrip.pop(s + 1))

        for th in pending:
            th()
    return nc


def _make_core_inputs(q, k, v, Wq, Wk, Wv, Wo, core, cache):
    bf = ml_dtypes.bfloat16
    if "qT" not in cache:
        cache["qT"] = np.ascontiguousarray(q.T).astype(bf)
        cache["kT"] = np.ascontiguousarray(k.T).astype(bf)
        cache["vT"] = np.ascontiguousarray(v.T).astype(bf)
    h0, h1 = 2 * core, 2 * core + 1
    return {
        "qT": cache["qT"],
        "kT": cache["kT"],
        "vT": cache["vT"],
        "wq": np.concatenate([Wq[h0], Wq[h1]], axis=1).astype(bf),
        "wk": np.concatenate([Wk[h0], Wk[h1]], axis=1).astype(bf),
        "wv": np.concatenate([Wv[h0], Wv[h1]], axis=1).astype(bf),
        "wo": np.ascontiguousarray(Wo[:, P * core : P * (core + 1)].T).astype(bf),
        "seld": np.kron(
            np.eye(2, dtype=np.float32), np.ones((1, DK), np.float32)
        ).astype(bf),
    }


_NC = None
last_results = None  # BassKernelResults of the most recent run (for profiling)


def _get_nc():
    global _NC
    if _NC is None:
        nc = bass.Bass("TRN2", target_bir_lowering=False, debug=False)
        _build_mha(nc)
        _split_excess_waits(nc)
        _NC = nc
    return _NC


def kernel(q, k, v, Wq, Wk, Wv, Wo, **run_kwargs):
    """Full-input MHA forward. Shards over 8 NeuronCores (2 heads each),
    runs the Bass kernel, and all-reduces the per-core partial outputs."""
    from concourse.bass_utils import run_bass_kernel_spmd

    global last_results
    q = np.asarray(q, np.float32)
    k = np.asarray(k, np.float32)
    v = np.asarray(v, np.float32)
    Wq = np.asarray(Wq, np.float32)
    Wk = np.asarray(Wk, np.float32)
    Wv = np.asarray(Wv, np.float32)
    Wo = np.asarray(Wo, np.float32)

    nc = _get_nc()
    cache = {}
    in_maps = [
        _make_core_inputs(q, k, v, Wq, Wk, Wv, Wo, c, cache) for c in range(NCORES)
    ]
    res = run_bass_kernel_spmd(
        nc, in_maps, core_ids=list(range(NCORES)), **run_kwargs
    )
    last_results = res
    y = res.results[0]["y"].astype(np.float32)
    for c in range(1, NCORES):
        y += res.results[c]["y"]
    return y



# revision 15
# speedup vs baseline: 1.2032x; 1.0093x over previous
"""Multi-head attention (S=4096, D=1024, H=16, dk=dv=64) on 8 trn2 NeuronCores.

Sharding: tensor-parallel over heads — 2 heads per core. Each core receives
the (host-transposed, bf16-cast) activations plus its two heads' projection
weights and its 128-column slice of Wo, computes its heads' attention and a
partial output product y_c = hc @ Wo[:, c-slice].T, and the host all-reduces
the 8 partials (the "row-shard W_o and all-reduce" variant, with the
all-reduce in the gather step).

Per-core Bass/Tile kernel (engines execute their streams in order, so the
program is software-pipelined by emission order):
  - Projections qhT/khT [128,S] (head A on partitions 0-63, head B on 64-127)
    and vh tiles [sk, dv]+ones-column (PE-transposed), streamed per 512-wide
    strip and interleaved into strip 0's attention loop (DMA-paced).
  - Attention, ACT-paced steady state: per sk tile, scores^T for both heads
    concurrently (PE row groups at base partitions 0/64), one Exp over the
    [128,1024] PSUM pair (scale=1/8; no max subtraction — scores are O(1)),
    two PV accumulations out^T[dv+1, sq]; the ones column yields the softmax
    denominator in row 64.
  - Per-strip epilogue, drip-fed into the NEXT strip's loop: one reciprocal
    over both denominator rows (adjacent partitions via an SBUF->SBUF DMA
    partition move), K=2 selector-matmul broadcast across 64 partitions
    (DVE cannot cross partitions), in-place normalize, head B shifted under
    head A by DMA, then single K=128 output-projection matmuls.

Matmul operands are bf16 (full PE rate; fp32 is 1/4 rate and fp32r is an
e11m8 format with the same 8-bit mantissa). PSUM accumulation is fp32; exp
input is exact fp32; denominators stay fp32 through the reciprocal.
PSUM (8 banks): qk pairs 2x[128,1024]=4, pv 2x[128,512]=2, post 2x[128,512]=2.
"""

import contextlib
import sys

if "/opt/trn_rl_repo" not in sys.path:
    sys.path.insert(0, "/opt/trn_rl_repo")

import numpy as np
import ml_dtypes

import concourse.bass as bass
import concourse.mybir as mybir
import concourse.tile as tile
from concourse.masks import make_identity

F32 = mybir.dt.float32
BF16 = mybir.dt.bfloat16
EXP = mybir.ActivationFunctionType.Exp
MULT = mybir.AluOpType.mult

S, D, DK, P, W = 4096, 1024, 64, 128, 512
NS = S // W      # 8 sq strips
NT = S // P      # 32 sk tiles
NDC = D // P     # 8 contraction chunks for projections
TPW = W // P     # sk tiles per strip (4)
SCALE = 0.125    # 1/sqrt(DK)
NCORES = 8


def _split_excess_waits(nc, max_waits=1, max_waits_evsem=2):
    """The walrus build in this container rejects instructions carrying more
    than ~2 sync-wait commands; Tile's exit drain aggregates one wait per live
    semaphore onto single instructions. Split the excess onto preceding NoOps
    on the same engine (engine streams are in-order, so semantics hold)."""
    for fn in nc.m.functions:
        for blk in fn.blocks:
            new_insts = []
            for inst in blk.instructions:
                si = getattr(inst, "sync_info", None)
                lim = (
                    max_waits_evsem
                    if isinstance(inst, mybir.InstEventSemaphore)
                    else max_waits
                )
                if si is not None and si.on_wait and len(si.on_wait) > lim:
                    waits = list(si.on_wait)
                    for w in waits[:-lim]:
                        new_insts.append(
                            mybir.InstNoOp(
                                name=nc.get_next_instruction_name(),
                                engine=inst.engine,
                                bass_nofuse=True,
                                sync_info=mybir.SyncInfo(on_wait=[w], on_update=[]),
                            )
                        )
                    si.on_wait = waits[-lim:]
                new_insts.append(inst)
            blk.instructions = new_insts


def _build_mha(nc: bass.Bass):
    qT = nc.dram_tensor("qT", [D, S], BF16, kind="ExternalInput")
    kT = nc.dram_tensor("kT", [D, S], BF16, kind="ExternalInput")
    vT = nc.dram_tensor("vT", [D, S], BF16, kind="ExternalInput")
    wq = nc.dram_tensor("wq", [D, P], BF16, kind="ExternalInput")
    wk = nc.dram_tensor("wk", [D, P], BF16, kind="ExternalInput")
    wv = nc.dram_tensor("wv", [D, P], BF16, kind="ExternalInput")
    wo = nc.dram_tensor("wo", [P, D], BF16, kind="ExternalInput")
    seld = nc.dram_tensor("seld", [2, 2 * DK], BF16, kind="ExternalInput")
    y = nc.dram_tensor("y", [S, D], BF16, kind="ExternalOutput")

    qT3 = qT.rearrange("(o p) s -> p o s", p=P)
    kT3 = kT.rearrange("(o p) s -> p o s", p=P)
    vT3 = vT.rearrange("(o p) s -> p o s", p=P)
    wq3 = wq.rearrange("(o p) m -> p o m", p=P)
    wk3 = wk.rearrange("(o p) m -> p o m", p=P)
    wv3 = wv.rearrange("(o p) m -> p o m", p=P)

    with tile.TileContext(nc) as tc, contextlib.ExitStack() as ctx:
        static = ctx.enter_context(tc.tile_pool(name="static", bufs=1))
        xpool = ctx.enter_context(tc.tile_pool(name="x", bufs=8))
        vtmp = ctx.enter_context(tc.tile_pool(name="vtmp", bufs=2))
        ptp = ctx.enter_context(tc.tile_pool(name="pt", bufs=6))
        recp = ctx.enter_context(tc.tile_pool(name="rec", bufs=2))
        ystage = ctx.enter_context(tc.tile_pool(name="ystage", bufs=4))
        qk_ps = ctx.enter_context(tc.tile_pool(name="qkps", bufs=2, space="PSUM"))
        pv_ps = ctx.enter_context(tc.tile_pool(name="pvps", bufs=2, space="PSUM"))
        post_ps = ctx.enter_context(tc.tile_pool(name="postps", bufs=2, space="PSUM"))

        wq_sb = static.tile([P, NDC, P], BF16, tag="wq")
        wk_sb = static.tile([P, NDC, P], BF16, tag="wk")
        wv_sb = static.tile([P, NDC, P], BF16, tag="wv")
        wo_sb = static.tile([P, D], BF16, tag="wo")
        ident = static.tile([P, P], BF16, tag="ident")
        sel = static.tile([DK + 2, 2 * DK], BF16, tag="sel")
        khT = static.tile([P, S], BF16, tag="khT")
        qhT = static.tile([P, S], BF16, tag="qhT")
        vh = static.tile([P, NT, 2 * DK + 2], BF16, tag="vh")
        hc = static.tile([P, S], BF16, tag="hc")
        tmpb_pool = recp

        nc.sync.dma_start(wq_sb[:], wq3)
        nc.sync.dma_start(wk_sb[:], wk3)
        nc.sync.dma_start(wv_sb[:], wv3)
        make_identity(nc, ident[:])
        # HAM warm-up: ~4us of dummy PE work (no DMA dependency) so the
        # projections hit the array already at 2.4 GHz.
        warm = post_ps.tile([P, W], F32, tag="post")
        for _ in range(40):
            nc.tensor.matmul(warm[:, 0:P], ident[:], ident[:], start=True, stop=True)

        def one_proj(jw, w_sb, src3, dst):
            def _th():
                xx = xpool.tile([P, NDC, W], BF16, tag="xs")
                nc.sync.dma_start(xx[:], src3[:, :, jw])
                pp = post_ps.tile([P, W], F32, tag="post")
                for c in range(NDC):
                    nc.tensor.matmul(
                        pp[:], w_sb[:, c, :], xx[:, c, :],
                        start=(c == 0), stop=(c == NDC - 1),
                    )
                nc.vector.tensor_copy(dst, pp[:])
            return _th

        def proj_thunks(j):
            """Projection strip j as 4 thunks: q, k, v, v-transposes."""
            jw = slice(j * W, (j + 1) * W)
            vts = vtmp.tile([P, W], BF16, tag="vts")

            def _trs():
                for i in range(TPW):
                    t = j * TPW + i
                    ptr = post_ps.tile([P, P], BF16, tag="post")
                    nc.tensor.transpose(ptr[:], vts[:, i * P : (i + 1) * P], ident[:])
                    # ptr rows = sk; cols 0:64 head A dv, 64:128 head B dv
                    nc.vector.tensor_copy(vh[:, t, 0:DK], ptr[:, 0:DK])
                    nc.vector.tensor_copy(
                        vh[:, t, DK + 1 : 2 * DK + 1], ptr[:, DK : 2 * DK]
                    )
                    nc.gpsimd.memset(vh[:, t, DK : DK + 1], 1.0)
                    nc.gpsimd.memset(vh[:, t, 2 * DK + 1 : 2 * DK + 2], 1.0)

            return [
                one_proj(jw, wq_sb, qT3, qhT[:, jw]),
                one_proj(jw, wk_sb, kT3, khT[:, jw]),
                one_proj(jw, wv_sb, vT3, vts[:]),
                _trs,
            ]

        def emit_proj(j):
            for th in proj_thunks(j):
                th()

        def make_epilogue(s, rec, tmpb):
            """Deferred post-softmax work for strip s, drip-fed into the next
            strip's attention loop (fills PE slack under the exp pace)."""
            cw = slice(s * W, (s + 1) * W)
            thunks = []

            def norm_a():
                bc = post_ps.tile([P, W], F32, tag="post")
                nc.tensor.matmul(
                    bc[0:DK, :], sel[DK : DK + 2, 0:DK], rec[DK : DK + 2, :],
                    start=True, stop=True,
                )
                nc.vector.tensor_tensor(
                    hc[0:DK, cw], hc[0:DK, cw], bc[0:DK, :], op=MULT
                )

            def norm_b():
                bc = post_ps.tile([P, W], F32, tag="post")
                nc.tensor.matmul(
                    bc[0:DK, :], sel[DK : DK + 2, DK : 2 * DK], rec[DK : DK + 2, :],
                    start=True, stop=True,
                )
                nc.vector.tensor_tensor(
                    tmpb[0:DK, :], tmpb[0:DK, :], bc[0:DK, :], op=MULT
                )
                nc.sync.dma_start(hc[DK:P, cw], tmpb[0:DK, :])

            thunks.append(norm_a)
            thunks.append(norm_b)

            def proj_out(i, oh):
                def _th():
                    sq = s * TPW + i
                    py = post_ps.tile([P, W], F32, tag="post")
                    nc.tensor.matmul(
                        py[:],
                        hc[:, sq * P : (sq + 1) * P],
                        wo_sb[:, oh * W : (oh + 1) * W],
                        start=True, stop=True,
                    )
                    ys = ystage.tile([P, W], BF16, tag="ys")
                    # final strip: exp stream is over, ScalarE is idle — let it
                    # share the PSUM evacuations so the tail chain is shorter
                    if s == NS - 1 and oh == 1:
                        nc.scalar.copy(ys[:], py[:])
                    else:
                        nc.vector.tensor_copy(ys[:], py[:])
                    nc.sync.dma_start(
                        y[sq * P : (sq + 1) * P, oh * W : (oh + 1) * W], ys[:]
                    )
                return _th

            for i in range(TPW):
                for oh in range(2):
                    thunks.append(proj_out(i, oh))
            return thunks

        # ---- main software-pipelined loop ----
        emit_proj(0)
        nc.sync.dma_start(wo_sb[:], wo[:])
        nc.sync.dma_start(sel[DK : DK + 2, :], seld[:])
        # per-strip projection thunks in dependency order; q(1) stays in
        # strip 0's drip (its consumer starts next strip), q(2..7) drip one
        # per strip s-1 instead of piling onto strip 0's PE
        projq = []
        qdrip = {}
        for j in range(1, NS):
            q_th, k_th, v_th, trs_th = proj_thunks(j)
            projq += [k_th, v_th, trs_th]
            if j == 1:
                projq.append(q_th)
            else:
                qdrip[j - 1] = q_th

        def emit_qk(s, t):
            """Scores^T for both heads of tile t against sq strip s."""
            scw = slice(s * W, (s + 1) * W)
            qk = qk_ps.tile([P, 2 * W], F32, tag="qk")
            nc.tensor.matmul(
                qk[0:P, 0:W],
                khT[0:DK, t * P : (t + 1) * P], qhT[0:DK, scw],
                start=True, stop=True,
            )
            nc.tensor.matmul(
                qk[0:P, W : 2 * W],
                khT[DK:P, t * P : (t + 1) * P], qhT[DK:P, scw],
                start=True, stop=True,
            )
            return qk

        pending = []
        qk_cur = emit_qk(0, 0)
        for s in range(NS):
            cw = slice(s * W, (s + 1) * W)
            pva = pv_ps.tile([P, W], F32, tag="pv")
            pvb = pv_ps.tile([P, W], F32, tag="pv")
            for t in range(NT):
                if s == 0 and projq:
                    projq.pop(0)()
                pt = ptp.tile([P, 2 * W], BF16, tag="pt")
                nc.scalar.activation(pt[:], qk_cur[:], EXP, scale=SCALE)
                # one-window QK lookahead: emit the NEXT tile's scores before
                # this tile's PV so the strip-boundary pipeline never refills
                if t + 1 < NT:
                    qk_cur = emit_qk(s, t + 1)
                elif s + 1 < NS:
                    qk_cur = emit_qk(s + 1, 0)
                nc.tensor.matmul(
                    pva[0 : DK + 1, :],
                    vh[:, t, 0 : DK + 1], pt[:, 0:W],
                    start=(t == 0), stop=(t == NT - 1),
                )
                nc.tensor.matmul(
                    pvb[0 : DK + 1, :],
                    vh[:, t, DK + 1 : 2 * DK + 2], pt[:, W : 2 * W],
                    start=(t == 0), stop=(t == NT - 1),
                )
                if pending and t >= 16 and t % 2 == 0:
                    pending.pop(0)()
                    if t == NT - 2:
                        while pending:
                            pending.pop(0)()

            # strip boundary: evacuate PSUM fast (head A + head B + the two
            # denominator rows, B's moved to the adjacent partition by DMA),
            # then one reciprocal for both heads — all off the exp path.
            # evacuate the big pv regions FIRST so the next strip's PV can
            # claim the banks before the slow reciprocal occupies the DVE
            rs = recp.tile([P, W], F32, tag="rs")
            tmpb = tmpb_pool.tile([DK, W], BF16, tag="tmpb")
            nc.vector.tensor_copy(hc[0:DK, cw], pva[0:DK, :])
            nc.vector.tensor_copy(tmpb[0:DK, :], pvb[0:DK, :])
            nc.vector.tensor_copy(rs[DK : DK + 1, :], pva[DK : DK + 1, :])
            rbt = recp.tile([P, W], F32, tag="rbt")
            nc.vector.tensor_copy(rbt[DK : DK + 1, :], pvb[DK : DK + 1, :])
            nc.sync.dma_start(rs[DK + 1 : DK + 2, :], rbt[DK : DK + 1, :])
            rec = recp.tile([P, W], BF16, tag="rec")
            with nc.allow_low_precision(
                reason="bf16 softmax denominators feed a bf16 matmul broadcast"
            ):
                nc.vector.reciprocal(rec[DK : DK + 2, :], rs[DK : DK + 2, :])

            assert not pending
            pending = make_epilogue(s, rec, tmpb)
            if (s + 1) in qdrip:
                pending.insert(0, qdrip.pop(s + 1))

        for th in pending:
            th()
    return nc


def _make_core_inputs(q, k, v, Wq, Wk, Wv, Wo, core, cache):
    bf = ml_dtypes.bfloat16
    if "qT" not in cache:
        cache["qT"] = np.ascontiguousarray(q.T).astype(bf)
        cache["kT"] = np.ascontiguousarray(k.T).astype(bf)
        cache["vT"] = np.ascontiguousarray(v.T).astype(bf)
    h0, h1 = 2 * core, 2 * core + 1
    return {
        "qT": cache["qT"],
        "kT": cache["kT"],
        "vT": cache["vT"],
        "wq": np.concatenate([Wq[h0], Wq[h1]], axis=1).astype(bf),
        "wk": np.concatenate([Wk[h0], Wk[h1]], axis=1).astype(bf),
        "wv": np.concatenate([Wv[h0], Wv[h1]], axis=1).astype(bf),
        "wo": np.ascontiguousarray(Wo[:, P * core : P * (core + 1)].T).astype(bf),
        "seld": np.kron(
            np.eye(2, dtype=np.float32), np.ones((1, DK), np.float32)
        ).astype(bf),
    }


_NC = None
last_results = None  # BassKernelResults of the most recent run (for profiling)


def _get_nc():
    global _NC
    if _NC is None:
        nc = bass.Bass("TRN2", target_bir_lowering=False, debug=False)
        _build_mha(nc)
        _split_excess_waits(nc)
        _NC = nc
    return _NC


def kernel(q, k, v, Wq, Wk, Wv, Wo, **run_kwargs):
    """Full-input MHA forward. Shards over 8 NeuronCores (2 heads each),
    runs the Bass kernel, and all-reduces the per-core partial outputs."""
    from concourse.bass_utils import run_bass_kernel_spmd

    global last_results
    q = np.asarray(q, np.float32)
    k = np.asarray(k, np.float32)
    v = np.asarray(v, np.float32)
    Wq = np.asarray(Wq, np.float32)
    Wk = np.asarray(Wk, np.float32)
    Wv = np.asarray(Wv, np.float32)
    Wo = np.asarray(Wo, np.float32)

    nc = _get_nc()
    cache = {}
    in_maps = [
        _make_core_inputs(q, k, v, Wq, Wk, Wv, Wo, c, cache) for c in range(NCORES)
    ]
    res = run_bass_kernel_spmd(
        nc, in_maps, core_ids=list(range(NCORES)), **run_kwargs
    )
    last_results = res
    y = res.results[0]["y"].astype(np.float32)
    for c in range(1, NCORES):
        y += res.results[c]["y"]
    return y

